# revision 28
# baseline (speedup 1.0000x reference)
"""Causal multi-head attention block (B=2, T=2048, C=1024, H=16) on 8 TRN2
NeuronCores.

Sharding (v4): 2D batch x head-group.  Core r = 4*g + i (g = batch, i =
group rank) owns heads [4i, 4i+4) of batch g, i.e. feature rows
[256i, 256i+256) of Wq/Wk/Wv, and output rows [256i, 256i+256) of the
final projection for batch g.  The y AllGather then runs inside each
4-core group (output 4 MB instead of 8 MB, and the two groups' gathers
run concurrently on separate dies) and each core only loads its own
batch's activations (4 MB instead of 8 MB).

On-device everything is computed feature-major (transposed) so the
TensorEngine contraction axis sits on SBUF partitions, and the softmax
denominator comes free via a ones-column appended to V:

  qT/kT/vT [128, ft, 2048] = W_shard @ x^T   (2 feature tiles of 128 =
                                              2 head-pairs)
  ST tile [128k, 512q] = kT_rows.T @ qT_rows (contract d=64)
  causal mask: matmul(ident, mneg) adds a -1e9 strictly-lower triangle
      into the St PSUM group on diagonal blocks
  PT = exp(ST / sqrt(d))     (logits ~N(0,1); no max subtraction)
  yT [65, 512] += [v | 1].T @ PT             (row 64 = denominator)
  y  = yT[0:64] * partition_broadcast(recip(yT[64]))

Performance structure (from perfetto/HAM analysis of v1-v3):
- The exp stream on ACT (~1.15us per 1024-col ACTIVATE, 80 of them) is
  the kernel's clock; the PE must stay dense to hold HAM at K=8/8.
  All projections beyond the very first q/k/v tile and the whole
  O-projection are split into single-matmul filler items popped between
  the exp-gated attention pairs.
- The scalar (ACT) queue carries only the exps plus 3 batched xT input
  DMA issues at t=0.  Large dma_starts fan across all 16 SDMA engines.
- The collective stream is serial per core and unavailable for the
  first ~60us (comm-stack init); each op also starts ~15-30us after its
  trigger.  Gathers are therefore asymmetric -- {jq0+jq1}, {jq2},
  {jq3} -- and O-proj chunks only enter the filler once their gather
  can physically have completed; the last small gather bounds the tail.
- PSUM tags: st 2x2 banks, yt 2x1, aux 2x1 (proj/O-proj/transpose/warm).

Inputs are bf16 (host-side cast); accumulation is f32 in PSUM; the
output shard is written bf16 and upcast to f32 on the host.
"""

import numpy as np
import ml_dtypes

import concourse.bacc as bacc
import concourse.mybir as mybir
import concourse.tile as tile
from concourse.bass_utils import run_bass_kernel_spmd
from concourse.masks import make_identity

N_CORES = 8
B, T, C, H = 2, 2048, 1024, 16
D = 64                # head dim
GW = 4                # group width (cores per batch group)
HL = H // GW          # heads per core = 4
DL = HL * D           # local feature dim = 256
NFT = DL // 128       # feature tiles per core = 2
TL = T                # local tokens = one batch = 2048
P = 128
NCH = C // P          # 8 contraction chunks
QCH = 512             # q-chunk (moving free dim)
NQC = TL // QCH       # 4 q-chunks
NKT = TL // P         # 16 k-tiles
SCALE = 1.0 / np.sqrt(D)

BF = mybir.dt.bfloat16
F32 = mybir.dt.float32
AF = mybir.ActivationFunctionType

REPLICA_GROUPS = [[0, 1, 2, 3], [4, 5, 6, 7]]


def build_graph():
    nc = bacc.Bacc("TRN2", target_bir_lowering=False, debug=False)

    xT = nc.dram_tensor("xT", [C, TL], BF, kind="ExternalInput")
    # 4 weight shards pre-packed host-side into SBUF layout [p, w, ci, m]
    wall = nc.dram_tensor("wall", [P, 4 * NCH * DL], BF,
                          kind="ExternalInput")
    out = nc.dram_tensor("out", [DL, TL], BF, kind="ExternalOutput")

    with tile.TileContext(nc) as tc:
        with (
            tc.tile_pool(name="sb", bufs=1) as sb,
            tc.tile_pool(name="ps", bufs=1, space="PSUM") as ps,
            tc.tile_pool(name="dram", bufs=1, space="DRAM") as dram,
        ):
            # ---- input loads ----
            w_sb = sb.tile([P, 4 * NCH * DL], BF, name="w_sb")
            WCOLS = 4 * NCH * DL
            for pc in range(8):
                csl = slice(pc * (WCOLS // 8), (pc + 1) * (WCOLS // 8))
                nc.sync.dma_start(w_sb[:, csl], wall[:, csl])
            w4 = w_sb[:].rearrange("p (w a m) -> p w a m", w=4, a=NCH)

            xT_sb = sb.tile([P, NCH, TL], BF, name="xT_sb")
            xTr = xT[:, :].rearrange("(a p) t -> p a t", p=P)
            for s0, s1 in ((0, QCH), (QCH, 2 * QCH), (2 * QCH, TL)):
                nc.scalar.dma_start(xT_sb[:, :, s0:s1], xTr[:, :, s0:s1])

            ident = sb.tile([P, P], BF, name="ident")
            make_identity(nc, ident)
            # strictly-lower-triangular -1e9 (k > q)
            mneg = sb.tile([P, P], BF, name="mneg")
            nc.gpsimd.memset(mneg[:], 0.0)
            nc.gpsimd.affine_select(
                out=mneg[:], in_=mneg[:],
                compare_op=mybir.AluOpType.is_ge,
                fill=-1e9, base=0, channel_multiplier=-1, pattern=[[1, P]],
            )
            wsrc = sb.tile([P, QCH], BF, name="wsrc")
            nc.vector.memset(wsrc[:], 0.5)

            def warm(n):
                for _ in range(n):
                    wdst = ps.tile([P, QCH], F32, tag="aux", bufs=2,
                                   name="wdst")
                    nc.tensor.matmul(wdst[:], ident[:], wsrc[:],
                                     start=True, stop=True)

            warm(6)

            qT_sb = sb.tile([P, NFT, TL], BF, name="qT_sb")
            kT_sb = sb.tile([P, NFT, TL], BF, name="kT_sb")
            vT_sb = sb.tile([P, NFT, TL], BF, name="vT_sb")
            # v natural layout per 128-token tile:
            # [h0(64) | 1 | h1 | 1 | h2 | 1 | h3 | 1] -> 260 columns
            v_sb = sb.tile([P, NKT, HL * (D + 1)], BF, name="v_sb")
            nc.vector.memset(v_sb[:], 1.0)

            # ---- AllGather buffers ----
            # one gather per jq slot; the last slot's gather is split
            # per head-pair so its first half fires mid-slot and the
            # tail only waits on a 512KB gather
            ag_in = [dram.tile([DL, QCH], BF, name=f"ag_in{g}")
                     for g in range(NQC - 1)]
            ytf = [dram.tile([C, QCH], BF, name=f"ytf{g}")
                   for g in range(NQC - 1)]
            ag3 = [dram.tile([D, QCH], BF, name=f"ag3_{h}")
                   for h in range(HL)]
            ytf3 = [dram.tile([GW * D, QCH], BF, name=f"ytf3_{h}")
                    for h in range(HL)]

            # ---- work items ----
            def make_proj_items(tch, ft, w, dst):
                tsl = slice(tch * QCH, (tch + 1) * QCH)
                msl = slice(ft * P, (ft + 1) * P)
                state = {}
                items = []
                for ci in range(NCH):
                    def mm(ci=ci):
                        if ci == 0:
                            state['pj'] = ps.tile([P, QCH], F32, tag="aux",
                                                  bufs=2, name="pj")
                        nc.tensor.matmul(
                            state['pj'][:], w4[:, w, ci, msl],
                            xT_sb[:, ci, tsl],
                            start=(ci == 0), stop=(ci == NCH - 1),
                        )
                        if ci == NCH - 1:
                            nc.vector.tensor_copy(dst[:, ft, tsl],
                                                  state['pj'][:])
                    items.append(mm)
                return items

            def make_vtrans_item(ft, t32):
                def it():
                    tr = ps.tile([P, P], BF, tag="aux", bufs=2, name="tr")
                    nc.tensor.transpose(
                        tr[:], vT_sb[:, ft, t32 * P:(t32 + 1) * P], ident[:]
                    )
                    out_ap = v_sb[:, t32, :].rearrange(
                        "p (h x) -> p h x", h=HL
                    )[:, 2 * ft:2 * ft + 2, 0:D]
                    in_ap = tr[:].rearrange("p (h x) -> p h x", h=2)
                    nc.vector.tensor_copy(out_ap, in_ap)
                return it

            def proj_items(tch, ft):
                items = []
                for w, dst in ((0, qT_sb), (1, kT_sb), (2, vT_sb)):
                    items.extend(make_proj_items(tch, ft, w, dst))
                for t32 in range(tch * 4, tch * 4 + 4):
                    items.append(make_vtrans_item(ft, t32))
                return items

            def yf_load(c):
                yf = sb.tile([P, NCH, QCH], BF, tag="yf", bufs=2, name="yf")
                src = ytf[c][:, :].rearrange("(a p) t -> p a t", p=P)
                nc.sync.dma_start(yf[:, :, :], src)
                return yf

            def make_po_items(c, yf, last=False):
                c0 = c * QCH
                items = []
                for mb in range(NFT):
                    state = {}
                    for ci in range(NCH):
                        def mm(ci=ci, mb=mb, state=state):
                            if ci == 0:
                                state['po'] = ps.tile(
                                    [P, QCH], F32, tag="aux", bufs=2,
                                    name="po")
                            nc.tensor.matmul(
                                state['po'][:],
                                w4[:, 3, ci, mb * P:(mb + 1) * P],
                                yf[:, ci, :],
                                start=(ci == 0), stop=(ci == NCH - 1),
                            )
                            if ci == NCH - 1:
                                ob = sb.tile([P, QCH], BF, tag="ob",
                                             bufs=2, name="ob")
                                nc.vector.tensor_copy(ob[:], state['po'][:])
                                nsp = 4 if last else 1
                                w_ = QCH // nsp
                                for sp in range(nsp):
                                    nc.sync.dma_start(
                                        out[mb * P:(mb + 1) * P,
                                            c0 + sp * w_:c0 + (sp + 1) * w_],
                                        ob[:, sp * w_:(sp + 1) * w_],
                                    )
                        items.append(mm)
                return items

            # ---- filler machinery ----
            # FIFO of (key, fn); key = 2*tch + ft for projection items
            # (deadline: key 2s before slot s, 2s+1 before its head 2),
            # 99 for O-proj items (no deadline).
            filler = []

            def pop_filler(n):
                if not filler:
                    # never let the PE queue run dry: a gapless stream
                    # is what holds HAM at full clock
                    warm(1)
                    return
                for _ in range(min(n, len(filler))):
                    filler.pop(0)[1]()

            def drain_key(kmax):
                while any(k <= kmax for k, _ in filler):
                    filler.pop(0)[1]()

            def drain_filler():
                while filler:
                    filler.pop(0)[1]()

            # ---- attention ----
            def ag_fire(g):
                nc.gpsimd.collective_compute(
                    "AllGather", mybir.AluOpType.bypass,
                    replica_groups=REPLICA_GROUPS,
                    ins=[ag_in[g][:]], outs=[ytf[g][:]],
                )

            def ag_fire3(h):
                nc.gpsimd.collective_compute(
                    "AllGather", mybir.AluOpType.bypass,
                    replica_groups=REPLICA_GROUPS,
                    ins=[ag3[h][:]], outs=[ytf3[h][:]],
                )

            def finish_head(jq, h, yt, den):
                bc = sb.tile([D, QCH], F32, tag="bc", bufs=3, name="bc")
                nc.gpsimd.partition_broadcast(bc[:], den[:])
                rcp = sb.tile([D, QCH], F32, tag="rcp", bufs=3, name="rcp")
                scr = sb.tile([D, QCH], F32, tag="scr", bufs=3, name="scr")
                nc.vector.reciprocal_approx_accurate(
                    rcp[:], bc[:], scratch=scr[:]
                )
                yn = sb.tile([D, QCH], BF, tag="yn", bufs=4, name="yn")
                nc.vector.tensor_mul(yn[:], yt[0:D, :], rcp[:])
                if jq < NQC - 1:
                    nc.gpsimd.dma_start(
                        ag_in[jq][h * D:(h + 1) * D, :], yn[:, :]
                    )
                else:
                    # last slot: per-head gather, fired immediately, so
                    # the tail only waits on the final 256KB gather
                    nc.gpsimd.dma_start(ag3[h][:, :], yn[:, :])
                    ag_fire3(h)

            def attn_slot(s, per_pair, head_hook=None):
                # one-deep software pipeline over the slot's pair
                # stream: pair i's AV matmuls are emitted after pair
                # i+1's scores+exp, so the PE never waits on ACT (the
                # exp-wait was holding HAM at half clock in v5)
                nkt = 4 * s + 4
                npr = nkt // 2
                q0 = s * QCH
                prev = None

                def emit_av(pv):
                    h, pr, pt, yt = pv
                    for half in range(2):
                        kt = 2 * pr + half
                        qv = max(kt - 4 * s, 0) * P
                        nc.tensor.matmul(
                            yt[:, qv:QCH],
                            v_sb[:, kt, h * (D + 1):(h + 1) * (D + 1)],
                            pt[:, half * QCH + qv:(half + 1) * QCH],
                            start=(kt == 0), stop=(kt == nkt - 1),
                        )
                    if pr == npr - 1:
                        den = sb.tile([1, QCH], F32, tag="den", bufs=4,
                                      name="den")
                        nc.vector.tensor_copy(den[:], yt[D:D + 1, :])
                        finish_head(s, h, yt, den)

                for h in range(HL):
                    if h == 2:
                        drain_key(2 * s + 1)
                    if head_hook:
                        head_hook(h)
                    th, hr = h // 2, h % 2
                    rsl = slice(hr * D, (hr + 1) * D)
                    yt = ps.tile([D + 1, QCH], F32, tag="yt", bufs=2,
                                 name="yt")
                    for pr in range(npr):
                        st = ps.tile([P, 2 * QCH], F32, tag="st", bufs=2,
                                     name="st")
                        pt = sb.tile([P, 2 * QCH], BF, tag="pt", bufs=4,
                                     name="pt")
                        for half in range(2):
                            kt = 2 * pr + half
                            k0 = kt * P
                            i = kt - 4 * s
                            qv = max(i, 0) * P
                            ssl = slice(half * QCH + qv, (half + 1) * QCH)
                            nc.tensor.matmul(
                                st[:, ssl],
                                kT_sb[rsl, th, k0:k0 + P],
                                qT_sb[rsl, th, q0 + qv:q0 + QCH],
                                start=True, stop=(i < 0),
                            )
                            if i >= 0:
                                nc.tensor.matmul(
                                    st[:, half * QCH + qv:
                                       half * QCH + qv + P],
                                    ident[:], mneg[:],
                                    start=False, stop=True,
                                )
                        qv0 = max(2 * pr - 4 * s, 0) * P
                        nc.scalar.activation(
                            pt[:, qv0:], st[:, qv0:], AF.Exp,
                            scale=float(SCALE)
                        )
                        # filler BEFORE the AV matmuls: the AV release
                        # sem rides the ACT queue's next sem-op, so the
                        # PE needs independent work at the queue head
                        # while that clears
                        pop_filler(per_pair)
                        if prev is not None:
                            emit_av(prev)
                        prev = (h, pr, pt, yt)
                emit_av(prev)

            # ---- prologue: tch0 feature-tile 0 only ----
            for w, dst in ((0, qT_sb), (1, kT_sb), (2, vT_sb)):
                pj = ps.tile([P, QCH], F32, tag="aux", bufs=2, name="pj")
                for ci in range(NCH):
                    nc.tensor.matmul(
                        pj[:], w4[:, w, ci, 0:P], xT_sb[:, ci, 0:QCH],
                        start=(ci == 0), stop=(ci == NCH - 1),
                    )
                nc.vector.tensor_copy(dst[:, 0, 0:QCH], pj[:])
            for t32 in range(4):
                make_vtrans_item(0, t32)()

            filler.extend((1, it) for it in proj_items(0, 1))
            filler.extend((2, it) for it in proj_items(1, 0))
            filler.extend((3, it) for it in proj_items(1, 1))

            # ---- main loop over jq slots ----
            # O-proj chunk c enters the filler only once gather c has
            # physically completed (fired at end of slot c; ~25us wall)
            PO_AT = {(2, 2): (0,), (3, 1): (1,), (3, 3): (2,)}

            def append_po(chunks):
                for c in chunks:
                    yf = yf_load(c)
                    filler.extend(
                        (99, it) for it in make_po_items(c, yf))

            for s in range(NQC):
                if s >= 1:
                    drain_key(2 * s)
                if s + 2 <= NQC - 1:
                    # tch s+2 projections join the filler at slot s
                    filler.extend((2 * (s + 2), it)
                                  for it in proj_items(s + 2, 0))
                    filler.extend((2 * (s + 2) + 1, it)
                                  for it in proj_items(s + 2, 1))
                npairs = 8 * (s + 1)
                due = sum(1 for k, _ in filler if k <= 2 * s + 2)
                per_pair = max(2, min(6, -(-due // npairs)))
                attn_slot(s, per_pair,
                          head_hook=lambda h, s=s: append_po(
                              PO_AT.get((s, h), ())))
                if s < NQC - 1:
                    ag_fire(s)

            # ---- tail: drain, then the parity-split last O-proj ----
            drain_filler()
            warm(6)
            yf3 = sb.tile([P, 2, 4, QCH], BF, tag="yf", bufs=2, name="yf3")
            po3_state = [{} for _ in range(NFT)]
            for parity in range(2):
                # head-pair (2*parity, 2*parity+1) fills partitions
                # 0-63 / 64-127 of the parity slice; rank r' lands at
                # mid index r' (= feature chunk ci 2*r'+parity)
                for hr in range(2):
                    h = 2 * parity + hr
                    src = ytf3[h][:, :].rearrange("(a p) t -> p a t", p=D)
                    nc.sync.dma_start(yf3[hr * D:(hr + 1) * D, parity, :, :],
                                      src)
                for mb in range(NFT):
                    state = po3_state[mb]
                    for r4 in range(4):
                        ci = 2 * r4 + parity
                        if parity == 0 and r4 == 0:
                            state['po'] = ps.tile([P, QCH], F32, tag="aux",
                                                  bufs=2, name="po")
                        nc.tensor.matmul(
                            state['po'][:],
                            w4[:, 3, ci, mb * P:(mb + 1) * P],
                            yf3[:, parity, r4, :],
                            start=(parity == 0 and r4 == 0),
                            stop=(parity == 1 and r4 == 3),
                        )
                        if parity == 1 and r4 == 3:
                            ob = sb.tile([P, QCH], BF, tag="ob", bufs=2,
                                         name="ob")
                            nc.vector.tensor_copy(ob[:], state['po'][:])
                            c0 = (NQC - 1) * QCH
                            for sp in range(4):
                                w_ = QCH // 4
                                nc.sync.dma_start(
                                    out[mb * P:(mb + 1) * P,
                                        c0 + sp * w_:c0 + (sp + 1) * w_],
                                    ob[:, sp * w_:(sp + 1) * w_],
                                )

    nc.finalize()
    return nc


_GRAPH = None


def _get_graph():
    global _GRAPH
    if _GRAPH is None:
        _GRAPH = build_graph()
    return _GRAPH


def prepare_in_maps(x, Wq, Wk, Wv, Wo):
    x = np.asarray(x, np.float32)
    Wq = np.asarray(Wq, np.float32)
    Wk = np.asarray(Wk, np.float32)
    Wv = np.asarray(Wv, np.float32)
    Wo = np.asarray(Wo, np.float32)

    bf = ml_dtypes.bfloat16
    xTh = [np.ascontiguousarray(x[g].T).astype(bf) for g in range(B)]
    in_maps = []
    for r in range(N_CORES):
        g, i = r // GW, r % GW
        sl = slice(i * DL, (i + 1) * DL)
        wall = np.empty((P, 4, NCH, DL), np.float32)
        for w, W in enumerate((Wq, Wk, Wv, Wo)):
            wall[:, w] = W[sl].T.reshape(NCH, P, DL).transpose(1, 0, 2)
        in_maps.append({
            "xT": xTh[g],
            "wall": np.ascontiguousarray(
                wall.reshape(P, 4 * NCH * DL)).astype(bf),
        })
    return in_maps


def assemble_output(results):
    outs = []
    for g in range(B):
        outT = np.concatenate(
            [np.asarray(results[GW * g + i]["out"], np.float32)
             for i in range(GW)], axis=0)  # [C, TL]
        outs.append(outT.T)
    return np.ascontiguousarray(np.stack(outs))  # [B, T, C]


def kernel(x, Wq, Wk, Wv, Wo):
    nc = _get_graph()
    in_maps = prepare_in_maps(x, Wq, Wk, Wv, Wo)
    res = run_bass_kernel_spmd(nc, in_maps, core_ids=list(range(N_CORES)))
    return assemble_output(res.results)
